# revision 1
# baseline (speedup 1.0000x reference)
"""AstroEconomicTransformer on 8 Trainium2 NeuronCores.

Sharding: 8-way sequence-parallel over the B*S = 2048 tokens (256 tokens
per core; cores 0-3 hold batch 0, cores 4-7 batch 1). Activations live
feature-major on chip (x^T: features on partitions, tokens on the free
dim), so every linear layer is a W^T-stationary matmul with the token
dim streaming. Attention needs the full sequence of K/V per batch
element, so each layer all-gathers K^T (feature-major) and token-major
V within each group of 4 cores; everything else (LayerNorm, FFN,
projections, softmax) is token-local. The final (B,S,1) output is
assembled on host from each core's (1,256) slice.

Attention per head: scores^T = k^T.T @ q^T (keys on partitions, queries
free), exp on the scalar engine with the 1/8 scale folded into the q
eviction and pbias as the activation bias (no max subtraction: scores
are O(1) by construction). Token-major V carries an all-ones column per
head, so the ctx matmul's row 64 is the softmax denominator; ctx is
normalized by a K=1 broadcast matmul of the reciprocal. The V bias is
folded on host into the attention output bias (bo_eff = bo + Wo @ bv,
exact because softmax rows sum to one).
"""

import numpy as np

B, S = 2, 1024
D, H, L, DFF = 1024, 16, 6, 4096
NM, NA, OUT = 10, 20, 1
HD = D // H
EPS = 1e-5

NCORES = 8
GPC = 4  # cores per batch group
T = (B * S) // NCORES  # 256 tokens per core
GROUPS = [[0, 1, 2, 3], [4, 5, 6, 7]]
P = 128
DT = D // P  # 8 feature tiles
FT = DFF // P  # 32 dff tiles
TT = T // P  # 2 token tiles per core
ST = S // P  # 8 key tiles per sequence
NPAIR = H // 2

VROW = H * (HD + 1)  # 1040: per-head ones-augmented v row
KELEM = D * T
VELEM = T * VROW

_RUNNER = None
REPS = 1  # timing amplification: repeat the whole forward pass


class _Cols:
    """Allocates columns in the (128, n) bias/constant matrix."""

    def __init__(self):
        self.cols = []

    def add(self, mat):  # mat: (128, n) -> first col index
        i = len(self.cols)
        self.cols.extend(np.asarray(mat, np.float32).T)
        return i

    def array(self):
        return np.stack(self.cols, axis=1).astype(np.float32)


def _group_kxm(wT, km, mm_):
    """(N, K, M) pre-transposed weight -> (N*mm_, P, km*P) SBUF tile images.

    Row (n, m) is the on-chip (128, km*128) tile whose column block k is the
    lhsT block wT[n, k*128:(k+1)*128, m*128:(m+1)*128] — a plain 2D DMA.
    """
    n = wT.shape[0]
    g = wT.reshape(n, km, P, mm_, P).transpose(0, 3, 2, 1, 4)
    return np.ascontiguousarray(g.reshape(n * mm_, P, km * P))


def _prep_host(inputs):
    f32 = np.float32
    g = {k: np.asarray(v, f32) for k, v in inputs.items()}

    cols = _Cols()
    idx = {}
    bemb = np.concatenate([g["bm"], g["ba"]])
    idx["bemb"] = cols.add(bemb.reshape(DT, P).T)
    for l in range(L):
        idx[f"bq{l}"] = cols.add((g["bq"][l] * 0.125).reshape(DT, P).T)
        idx[f"bk{l}"] = cols.add(g["bk"][l].reshape(DT, P).T)
        bo_eff = g["bo"][l] + g["Wo"][l] @ g["bv"][l]
        idx[f"bo{l}"] = cols.add(bo_eff.reshape(DT, P).T)
        idx[f"b1{l}"] = cols.add(g["b1"][l].reshape(FT, P).T)
        idx[f"b2{l}"] = cols.add(g["b2"][l].reshape(DT, P).T)
        idx[f"g1{l}"] = cols.add(g["ln1_g"][l].reshape(DT, P).T)
        idx[f"be1{l}"] = cols.add(g["ln1_b"][l].reshape(DT, P).T)
        idx[f"g2{l}"] = cols.add(g["ln2_g"][l].reshape(DT, P).T)
        idx[f"be2{l}"] = cols.add(g["ln2_b"][l].reshape(DT, P).T)
        idx[f"pb{l}"] = cols.add(np.tile(g["pbias"][l][None, :], (P, 1)))
    idx["gf"] = cols.add(g["lnf_g"].reshape(DT, P).T)
    idx["bef"] = cols.add(g["lnf_b"].reshape(DT, P).T)
    idx["bout"] = cols.add(np.full((P, 1), g["bout"][0], f32))
    idx["eps"] = cols.add(np.full((P, 1), EPS, f32))
    bcols = cols.array()
    idx["_nbc"] = bcols.shape[1]

    import ml_dtypes

    bf = lambda a: np.ascontiguousarray(a).astype(ml_dtypes.bfloat16)
    tr = lambda w: w.transpose(0, 2, 1)
    shared = {
        "bcols": bcols,
        "WmT": np.ascontiguousarray(g["Wm"].T),
        "WaT": np.ascontiguousarray(g["Wa"].T),
        "Wq_g": bf(_group_kxm(tr(g["Wq"]), DT, DT)),
        "Wk_g": bf(_group_kxm(tr(g["Wk"]), DT, DT)),
        "WvT": bf(tr(g["Wv"])),  # rhs-moving, row slabs
        "Wo_g": bf(_group_kxm(tr(g["Wo"]), DT, DT)),
        "W1_g": bf(_group_kxm(tr(g["W1"]), DT, FT)),
        "W2_g": bf(_group_kxm(tr(g["W2"]), FT, DT)),
        "WoutT": np.ascontiguousarray(g["Wout"].T),
        "onesb": np.ones((P, P), f32),
    }

    per_core = []
    peT_full = np.ascontiguousarray(g["pe"][0].T)
    for c in range(NCORES):
        b, chunk = c // GPC, c % GPC
        r0 = chunk * T
        per_core.append(
            {
                "mktT": np.ascontiguousarray(g["market_data"][b, r0 : r0 + T, :].T),
                "astT": np.ascontiguousarray(g["astro_data"][b, r0 : r0 + T, :].T),
                "peT": np.ascontiguousarray(peT_full[:, r0 : r0 + T]),
            }
        )
    return shared, per_core, idx


# ---------------------------------------------------------------- device kernel
def _build(idx):
    from contextlib import ExitStack

    import concourse.mybir as mybir
    import concourse.tile as tile
    from concourse import bacc

    dt = mybir.dt
    F32, F32R, BF16 = dt.float32, dt.float32r, dt.bfloat16
    AF = mybir.ActivationFunctionType
    ALU = mybir.AluOpType

    nc = bacc.Bacc("TRN2", debug=False, num_devices=NCORES)

    NBC = idx["_nbc"]
    GK = DT * P * P  # elements in one qkvo/W1 k-group (8 tiles)
    GK2 = FT * P * P  # elements in one W2 m-group (32 tiles)

    mktT = nc.declare_dram_parameter("mktT", [NM, T], F32R, isOutput=False)
    astT = nc.declare_dram_parameter("astT", [NA, T], F32R, isOutput=False)
    peT = nc.declare_dram_parameter("peT", [D, T], F32, isOutput=False)
    bcols_d = nc.declare_dram_parameter("bcols", [P, NBC], F32, isOutput=False)
    WmT = nc.declare_dram_parameter("WmT", [NM, D // 2], F32R, isOutput=False)
    WaT = nc.declare_dram_parameter("WaT", [NA, D // 2], F32R, isOutput=False)
    Wq_g = nc.declare_dram_parameter("Wq_g", [L * DT, P, DT * P], BF16, isOutput=False)
    Wk_g = nc.declare_dram_parameter("Wk_g", [L * DT, P, DT * P], BF16, isOutput=False)
    WvT = nc.declare_dram_parameter("WvT", [L, D, D], BF16, isOutput=False)
    Wo_g = nc.declare_dram_parameter("Wo_g", [L * DT, P, DT * P], BF16, isOutput=False)
    W1_g = nc.declare_dram_parameter("W1_g", [L * FT, P, DT * P], BF16, isOutput=False)
    W2_g = nc.declare_dram_parameter("W2_g", [L * DT, P, FT * P], BF16, isOutput=False)
    WoutT = nc.declare_dram_parameter("WoutT", [D, OUT], F32R, isOutput=False)
    ones_d = nc.declare_dram_parameter("onesb", [P, P], F32R, isOutput=False)
    y_out = nc.declare_dram_parameter("y", [1, T], F32, isOutput=True)

    k_in = [nc.dram_tensor(f"k_in{l}", [KELEM], F32) for l in range(L)]
    k_ag = [nc.dram_tensor(f"k_ag{l}", [GPC, KELEM], F32) for l in range(L)]
    v_in = [nc.dram_tensor(f"v_in{l}", [VELEM], F32) for l in range(L)]
    v_ag = [nc.dram_tensor(f"v_ag{l}", [GPC, VELEM], F32) for l in range(L)]

    with tile.TileContext(nc) as tc, ExitStack() as ctx:
        def pool(name, bufs, space="SBUF"):
            return ctx.enter_context(tc.tile_pool(name=name, bufs=bufs, space=space))

        singles = pool("singles", 1)
        xp = pool("xarr", 3)
        qp = pool("qarr", 1)
        cxp = pool("ctxarr", 1)
        hp = pool("harr", 1)
        xbp = pool("xbf", 3)  # bf16 shadows of the residual stream
        wp = pool("wrow", 6)  # qkvo/W1 k-groups + WvT row slabs (bf16)
        w2p = pool("w2grp", 2)  # (128,4096) bf16
        kvp = pool("kvloc", 2)
        app = pool("attn", 6)
        exp_p = pool("exparr", 3)
        sqp = pool("sqp", 2)
        bcp = pool("bcp", 2)
        lntp = pool("lntp", 2)
        sp = pool("small", 3)
        embp = pool("embp", 2)

        ps = pool("ps", 4, space="PSUM")
        psx = pool("psx", 4, space="PSUM")

        bc = singles.tile([P, NBC], F32)
        nc.sync.dma_start(bc[:], bcols_d[:])
        onesb = singles.tile([P, P], F32R)
        nc.sync.dma_start(onesb[:], ones_d[:])
        pe_sb = singles.tile([P, DT * T], F32)
        nc.sync.dma_start(pe_sb[:].rearrange("p (a t) -> p a t", t=T), peT[:].rearrange("(a p) t -> p a t", p=P))
        wout_sb = singles.tile([P, DT], F32R)
        nc.sync.dma_start(wout_sb[:].rearrange("p (a o) -> p a o", o=OUT), WoutT[:].rearrange("(a p) o -> p a o", p=P))

        def col(name, j=0, rows=P):
            return bc[0:rows, idx[name] + j : idx[name] + j + 1]

        def mm(out, lhsT, rhs, start, stop):
            nc.tensor.matmul(out, lhsT, rhs, start=start, stop=stop)

        for _rep in range(REPS):
            # ======================================================== embed
            x = xp.tile([P, DT * T], F32R, tag="xarr")
            in_sb = embp.tile([NA, 2 * T], F32R, tag="embin")
            nc.sync.dma_start(in_sb[0:NM, 0:T], mktT[:])
            nc.sync.dma_start(in_sb[0:NA, T : 2 * T], astT[:])
            wemb = embp.tile([NA, D // 2], F32R, tag="wemb")
            nc.sync.dma_start(wemb[0:NM, :], WmT[:])
            wemb2 = embp.tile([NA, D // 2], F32R, tag="wemb")
            nc.sync.dma_start(wemb2[:], WaT[:])
            for m in range(DT):
                pm = ps.tile([P, T], F32, tag="ps")
                if m < 4:
                    w, nin, toff, mo = wemb, NM, 0, m
                else:
                    w, nin, toff, mo = wemb2, NA, T, m - 4
                mm(pm[:], w[0:nin, mo * P : (mo + 1) * P], in_sb[0:nin, toff : toff + T],
                   start=True, stop=True)
                nc.vector.scalar_tensor_tensor(
                    x[:, m * T : (m + 1) * T], pm[:], col("bemb", m),
                    pe_sb[:, m * T : (m + 1) * T], ALU.add, ALU.add,
                )

            # ============================================================ helpers
            def load_group(pl, src_row, ntiles, tag):
                t = pl.tile([P, ntiles * P], BF16, tag=tag)
                nc.sync.dma_start(t[:], src_row)
                return t

            def shadow(src_arr, name):
                b = xbp.tile([P, DT * T], BF16, tag="xbf", name=name)
                for m in range(DT):
                    nc.scalar.copy(b[:, m * T : (m + 1) * T], src_arr[:, m * T : (m + 1) * T])
                return b

            def proj_fm(wg_d, row0, src, bias_fn, dst_fn, nm=DT, nk=DT):
                """Feature-major projection: dst[m] = act(W @ src + bias)."""
                for m in range(nm):
                    grp = load_group(w2p if nk == FT else wp, wg_d[row0 + m, :, :], nk,
                                     "w2grp" if nk == FT else "wrow")
                    pr = ps.tile([P, T], F32, tag="ps")
                    for kk in range(nk):
                        mm(pr[:], grp[:, kk * P : (kk + 1) * P],
                           src(kk), start=(kk == 0), stop=(kk == nk - 1))
                    dst_fn(m, pr, bias_fn(m))

            def layernorm(src_tiles, gname, bname, dst):
                s_ps = psx.tile([1, T], F32, tag="aux")
                s2_ps = psx.tile([1, T], F32, tag="aux")
                sqt = []
                for m in range(DT):
                    sq = sqp.tile([P, T], F32R, tag="sq")
                    nc.vector.tensor_mul(sq[:], src_tiles[m], src_tiles[m])
                    sqt.append(sq)
                    mm(s2_ps[:], onesb[:, 0:1], sq[:], start=(m == 0), stop=(m == DT - 1))
                for m in range(DT):
                    mm(s_ps[:], onesb[:, 0:1], src_tiles[m], start=(m == 0), stop=(m == DT - 1))
                mu = sp.tile([1, T], F32R, tag="stat1")
                nc.vector.tensor_scalar_mul(mu[:], s_ps[:], 1.0 / D)
                ex2 = sp.tile([1, T], F32, tag="stat1")
                nc.vector.tensor_scalar_mul(ex2[:], s2_ps[:], 1.0 / D)
                mu2 = sp.tile([1, T], F32, tag="stat1")
                nc.vector.tensor_mul(mu2[:], mu[:], mu[:])
                var = sp.tile([1, T], F32, tag="stat1")
                nc.vector.tensor_sub(var[:], ex2[:], mu2[:])
                sd = sp.tile([1, T], F32, tag="stat1")
                nc.scalar.activation(sd[:], var[:], AF.Sqrt, bias=col("eps", rows=1), scale=1.0)
                rs = sp.tile([1, T], F32R, tag="stat1")
                with nc.allow_low_precision(reason="fp32r feeds the broadcast matmul"):
                    nc.vector.reciprocal(rs[:], sd[:])
                mub_ps = ps.tile([P, T], F32, tag="ps")
                mm(mub_ps[:], onesb[0:1, :], mu[:], start=True, stop=True)
                rsb_ps = ps.tile([P, T], F32, tag="ps")
                mm(rsb_ps[:], onesb[0:1, :], rs[:], start=True, stop=True)
                mub = bcp.tile([P, T], F32, tag="bcast")
                nc.scalar.copy(mub[:], mub_ps[:])
                rsb = bcp.tile([P, T], F32, tag="bcast")
                nc.scalar.copy(rsb[:], rsb_ps[:])
                for m in range(DT):
                    t1 = lntp.tile([P, T], F32, tag="lnt")
                    nc.vector.tensor_sub(t1[:], src_tiles[m], mub[:])
                    t2 = lntp.tile([P, T], F32, tag="lnt")
                    nc.vector.tensor_mul(t2[:], t1[:], rsb[:])
                    nc.vector.tensor_scalar(
                        dst[:, m * T : (m + 1) * T], t2[:], col(gname, m), col(bname, m),
                        ALU.mult, ALU.add,
                    )

            # ============================================================ layers
            xb = shadow(x, "xb_emb")
            for l in range(L):
                # ---- k projection (feature-major) + AG
                def k_dst(m, pr, bias, l=l):
                    kt = kvp.tile([P, T], F32R, tag="kloc")
                    nc.vector.tensor_scalar_add(kt[:], pr[:], bias)
                    nc.gpsimd.dma_start(
                        k_in[l][:].bitcast(F32R).rearrange("(r t) -> r t", t=T)[m * P : (m + 1) * P, :],
                        kt[:],
                    )

                proj_fm(Wk_g, l * DT, lambda kk, xb=xb: xb[:, kk * T : (kk + 1) * T],
                        lambda m, l=l: col(f"bk{l}", m), k_dst)
                nc.gpsimd.collective_compute(
                    "AllGather", ALU.bypass, replica_groups=GROUPS,
                    ins=[k_in[l][:].opt()], outs=[k_ag[l][:].opt()],
                )

                # ---- v projection (token-major, ones-augmented) + AG
                vls = []
                for mt in range(TT):
                    vl = kvp.tile([P, VROW], F32R, tag="vloc")
                    vls.append(vl)
                for n in range(2):
                    pvs = [ps.tile([P, 512], F32, tag="ps", name=f"pv{l}_{n}_{mt}")
                           for mt in range(TT)]
                    for kk in range(DT):
                        slab = wp.tile([P, 512], BF16, tag="vslab")
                        nc.sync.dma_start(
                            slab[:], WvT[l, kk * P : (kk + 1) * P, n * 512 : (n + 1) * 512]
                        )
                        for mt in range(TT):
                            mm(pvs[mt][:], xb[:, kk * T + mt * P : kk * T + (mt + 1) * P],
                               slab[:], start=(kk == 0), stop=(kk == DT - 1))
                    for mt in range(TT):
                        vl3 = vls[mt][:].rearrange("p (h c) -> p h c", c=HD + 1)
                        nc.scalar.copy(vl3[:, n * 8 : (n + 1) * 8, 0:HD], pvs[mt][:])
                for mt in range(TT):
                    vl3 = vls[mt][:].rearrange("p (h c) -> p h c", c=HD + 1)
                    nc.gpsimd.dma_start(vl3[:, :, HD : HD + 1], ones_d[:, 0:H])
                    nc.gpsimd.dma_start(
                        v_in[l][:].bitcast(F32R).rearrange("(r t) -> r t", t=VROW)[mt * P : (mt + 1) * P, :],
                        vls[mt][:],
                    )
                nc.gpsimd.collective_compute(
                    "AllGather", ALU.bypass, replica_groups=GROUPS,
                    ins=[v_in[l][:].opt()], outs=[v_ag[l][:].opt()],
                )

                # ---- q projection (feature-major, pre-scaled by 1/8)
                q = qp.tile([P, DT * T], F32R, tag="qarr")

                def q_dst(m, pr, bias, q=q):
                    nc.vector.tensor_scalar(
                        q[:, m * T : (m + 1) * T], pr[:], 0.125, bias, ALU.mult, ALU.add
                    )

                proj_fm(Wq_g, l * DT, lambda kk, xb=xb: xb[:, kk * T : (kk + 1) * T],
                        lambda m, l=l: col(f"bq{l}", m), q_dst)

                # ---- attention, one head pair per q partition tile
                ctxa = cxp.tile([P, DT * T], BF16, tag="ctxarr")
                for p in range(NPAIR):
                    ea = exp_p.tile([P, ST * T], F32R, tag="exparr")
                    eb = exp_p.tile([P, ST * T], F32R, tag="exparr")
                    for c in range(GPC):
                        kpair = app.tile([P, T], F32R, tag="kpair")
                        nc.gpsimd.dma_start(
                            kpair[:],
                            k_ag[l][c, :].bitcast(F32R).rearrange("(r t) -> r t", t=T)[
                                p * P : (p + 1) * P, :
                            ],
                        )
                        for sub in range(TT):
                            tk = TT * c + sub
                            for h01 in range(2):
                                o = h01 * HD
                                pscore = ps.tile([P, T], F32, tag="ps")
                                mm(pscore[:], kpair[o : o + HD, sub * P : (sub + 1) * P],
                                   q[o : o + HD, p * T : (p + 1) * T], start=True, stop=True)
                                dst = ea if h01 == 0 else eb
                                nc.scalar.activation(
                                    dst[:, tk * T : (tk + 1) * T], pscore[:], AF.Exp,
                                    bias=col(f"pb{l}", 2 * p + h01), scale=1.0,
                                )
                    pctx_a = psx.tile([HD + 1, T], F32, tag="aux")
                    pctx_b = psx.tile([HD + 1, T], F32, tag="aux")
                    for tk in range(ST):
                        c, sub = tk // TT, tk % TT
                        vp_t = app.tile([P, 2 * (HD + 1)], F32R, tag="vp")
                        nc.gpsimd.dma_start(
                            vp_t[:],
                            v_ag[l][c, :].bitcast(F32R).rearrange("(r t) -> r t", t=VROW)[
                                sub * P : (sub + 1) * P,
                                2 * p * (HD + 1) : 2 * (p + 1) * (HD + 1),
                            ],
                        )
                        mm(pctx_a[:], vp_t[:, 0 : HD + 1], ea[:, tk * T : (tk + 1) * T],
                           start=(tk == 0), stop=(tk == ST - 1))
                        mm(pctx_b[:], vp_t[:, HD + 1 : 2 * (HD + 1)], eb[:, tk * T : (tk + 1) * T],
                           start=(tk == 0), stop=(tk == ST - 1))
                    for h01 in range(2):
                        pctx = pctx_a if h01 == 0 else pctx_b
                        rect = sp.tile([HD + 1, T], F32R, tag="rec")
                        rec = rect[HD : HD + 1, :]
                        with nc.allow_low_precision(reason="fp32r feeds the broadcast matmul"):
                            nc.vector.reciprocal(rec, pctx[HD : HD + 1, :])
                        pbc = ps.tile([HD, T], F32, tag="ps")
                        mm(pbc[:], onesb[HD : HD + 1, 0:HD], rec, start=True, stop=True)
                        bcsb = sp.tile([HD, T], F32, tag="bc64")
                        nc.scalar.copy(bcsb[:], pbc[:])
                        if h01 == 0:
                            nc.vector.tensor_mul(
                                ctxa[0:HD, p * T : (p + 1) * T], pctx[0:HD, :], bcsb[:]
                            )
                        else:
                            ctmp = sp.tile([HD, T], BF16, tag="ctmp")
                            nc.vector.tensor_mul(ctmp[:], pctx[0:HD, :], bcsb[:])
                            nc.gpsimd.dma_start(
                                ctxa[HD:P, p * T : (p + 1) * T], ctmp[:]
                            )

                # ---- out projection + residual + LN1
                x1p = xp.tile([P, DT * T], F32R, tag="xarr")

                def o_dst(m, pr, bias, x1p=x1p, x=x):
                    nc.vector.scalar_tensor_tensor(
                        x1p[:, m * T : (m + 1) * T], pr[:], bias,
                        x[:, m * T : (m + 1) * T], ALU.add, ALU.add,
                    )

                proj_fm(Wo_g, l * DT, lambda kk: ctxa[:, kk * T : (kk + 1) * T],
                        lambda m, l=l: col(f"bo{l}", m), o_dst)
                x1 = xp.tile([P, DT * T], F32R, tag="xarr")
                layernorm([x1p[:, m * T : (m + 1) * T] for m in range(DT)],
                          f"g1{l}", f"be1{l}", x1)
                x1b = shadow(x1, f"x1b_{l}")

                # ---- FFN
                harr = hp.tile([P, FT * T], BF16, tag="harr")

                def h_dst(mf, pr, bias, harr=harr):
                    nc.vector.tensor_scalar(
                        harr[:, mf * T : (mf + 1) * T], pr[:], bias, 0.0, ALU.add, ALU.max
                    )

                proj_fm(W1_g, l * FT, lambda kk, x1b=x1b: x1b[:, kk * T : (kk + 1) * T],
                        lambda mf, l=l: col(f"b1{l}", mf), h_dst, nm=FT)

                x2p = xp.tile([P, DT * T], F32R, tag="xarr")

                def y_dst(m, pr, bias, x2p=x2p, x1=x1):
                    nc.vector.scalar_tensor_tensor(
                        x2p[:, m * T : (m + 1) * T], pr[:], bias,
                        x1[:, m * T : (m + 1) * T], ALU.add, ALU.add,
                    )

                proj_fm(W2_g, l * DT, lambda kf: harr[:, kf * T : (kf + 1) * T],
                        lambda m, l=l: col(f"b2{l}", m), y_dst, nk=FT)
                x2 = xp.tile([P, DT * T], F32R, tag="xarr")
                layernorm([x2p[:, m * T : (m + 1) * T] for m in range(DT)],
                          f"g2{l}", f"be2{l}", x2)
                x = x2
                if l < L - 1:
                    xb = shadow(x, f"xb_{l + 1}")

            # ============================================================ head
            xf = xp.tile([P, DT * T], F32R, tag="xarr")
            layernorm([x[:, m * T : (m + 1) * T] for m in range(DT)], "gf", "bef", xf)
            pyf = psx.tile([1, T], F32, tag="aux")
            for m in range(DT):
                mm(pyf[:], wout_sb[:, m : m + 1], xf[:, m * T : (m + 1) * T],
                   start=(m == 0), stop=(m == DT - 1))
            ysb = sp.tile([1, T], F32, tag="stat1")
            nc.scalar.activation(ysb[:], pyf[:], AF.Identity,
                                 bias=col("bout", 0, rows=1), scale=1.0)
            nc.sync.dma_start(y_out[:], ysb[:])

    nc.compile()
    return nc


# ---------------------------------------------------------------- runner
_SHARED_NAMES = frozenset(
    ["bcols", "WmT", "WaT", "Wq_g", "Wk_g", "WvT", "Wo_g", "W1_g", "W2_g", "WoutT", "onesb"]
)


def _make_runner(nc):
    import jax
    import concourse.mybir as mybir
    from concourse import bass2jax
    from jax.sharding import Mesh, PartitionSpec
    from jax.experimental.shard_map import shard_map

    bass2jax.install_neuronx_cc_hook()

    partition_name = nc.partition_id_tensor.name if nc.partition_id_tensor else None
    in_names, out_names, out_avals = [], [], []
    for alloc in nc.m.functions[0].allocations:
        if not isinstance(alloc, mybir.MemoryLocationSet):
            continue
        name = alloc.memorylocations[0].name
        if alloc.kind == "ExternalInput":
            if name != partition_name:
                in_names.append(name)
        elif alloc.kind == "ExternalOutput":
            out_names.append(name)
            out_avals.append(
                jax.core.ShapedArray(tuple(alloc.tensor_shape), mybir.dt.np(alloc.dtype))
            )
    n_params = len(in_names)
    n_outs = len(out_avals)
    all_in = in_names + out_names + ([partition_name] if partition_name else [])
    donate = tuple(range(n_params, n_params + n_outs))

    def _body(*args):
        operands = list(args)
        if partition_name is not None:
            operands.append(bass2jax.partition_id_tensor())
        return tuple(
            bass2jax._bass_exec_p.bind(
                *operands,
                out_avals=tuple(out_avals),
                in_names=tuple(all_in),
                out_names=tuple(out_names),
                lowering_input_output_aliases=(),
                sim_require_finite=True,
                sim_require_nnan=True,
                nc=nc,
            )
        )

    from jax.sharding import NamedSharding

    devices = jax.devices()[:NCORES]
    mesh = Mesh(np.asarray(devices), ("core",))
    repl_sharding = NamedSharding(mesh, PartitionSpec(None))
    core_sharding = NamedSharding(mesh, PartitionSpec("core"))
    in_specs = tuple(
        PartitionSpec(None) if name in _SHARED_NAMES else PartitionSpec("core")
        for name in in_names
    ) + (PartitionSpec("core"),) * n_outs
    out_specs = (PartitionSpec("core"),) * n_outs
    sharded = jax.jit(
        shard_map(_body, mesh=mesh, in_specs=in_specs, out_specs=out_specs,
                  check_rep=False),
        donate_argnums=donate,
        keep_unused=True,
    )

    class Runner:
        def upload(self, shared, per_core):
            ins = []
            for name in in_names:
                if name in _SHARED_NAMES:
                    ins.append((np.asarray(shared[name]), repl_sharding))
                else:
                    ins.append(
                        (
                            np.concatenate(
                                [np.asarray(per_core[c][name])
                                 for c in range(NCORES)],
                                axis=0,
                            ),
                            core_sharding,
                        )
                    )
            self.in_dev = [jax.device_put(a, s) for a, s in ins]
            jax.block_until_ready(self.in_dev)

        def dispatch(self):
            zeros = [
                jax.device_put(
                    np.zeros((NCORES * av.shape[0], *av.shape[1:]), av.dtype),
                    core_sharding,
                )
                for av in out_avals
            ]
            return sharded(*self.in_dev, *zeros)

        def collect(self, out_arrs):
            return [
                {
                    name: np.asarray(out_arrs[i]).reshape(NCORES, *out_avals[i].shape)[c]
                    for i, name in enumerate(out_names)
                }
                for c in range(NCORES)
            ]

        def run(self):
            import jax as _jax

            out_arrs = self.dispatch()
            _jax.block_until_ready(out_arrs)
            return self.collect(out_arrs)

    return Runner()


def get_runner(inputs):
    """Build (once) and return the runner with inputs uploaded."""
    global _RUNNER
    shared, per_core, idx = _prep_host(inputs)
    if _RUNNER is None:
        nc = _build(idx)
        _RUNNER = _make_runner(nc)
    _RUNNER.upload(shared, per_core)
    return _RUNNER


def kernel(**inputs) -> np.ndarray:
    runner = get_runner(inputs)
    res = runner.run()
    out = np.zeros((B, S, OUT), np.float32)
    for c in range(NCORES):
        b, chunk = c // GPC, c % GPC
        out[b, chunk * T : (chunk + 1) * T, 0] = res[c]["y"][0]
    return out

